# revision 22
# baseline (speedup 1.0000x reference)
"""Multi-head attention (B=2, L=2048, D=1024, H=16) on 8 trn2 NeuronCores.

Sharding: core c -> batch b = c//4, head-group g = c%4 (4 heads each).
Each core projects Q/K/V for its 256-wide d_model slice, runs attention for
its 4 heads, then an AllGather inside each 4-core batch group collects the
per-head-group attention outputs (as X.T, concat-dim on partitions) and every
core computes a distinct 256-column shard of the final Wo projection (its
own Wo row-slice arrives as per-core input data, so all addressing is
static).  The host stacks the column shards.

Math notes (exactness-preserving simplifications):
  - K-projection bias bk adds a per-query constant to every score row and
    cancels exactly in softmax -> dropped on device.
  - V bias bv contributes (sum_j P[i,j]) * bv = bv after normalization, so
    y += bv @ Wo.T; it is folded into bo on the host (bo_eff = bo + Wo @ bv).
  - Scores never exceed ~|2.5| for these input scales -> softmax runs
    without max subtraction (mathematically identical).
Compute is bf16 into fp32 PSUM accumulation throughout.
"""

import sys

if "/opt/trn_rl_repo" not in sys.path:
    sys.path.insert(0, "/opt/trn_rl_repo")

import numpy as np

B, L_FULL, D, H = 2, 2048, 1024, 16
G = 4  # cores per batch group (head parallel)
NCORES = 8
HL = H // G  # heads per core = 4
DH = D // H  # 64
DG = D // G  # d_model slice per core = 256
P = 128
REPLICA_GROUPS = [[0, 1, 2, 3], [4, 5, 6, 7]]
SCALE = 1.0 / float(np.sqrt(DH))

_BUILD_CACHE = {}


def build_kernel(L=L_FULL, n_devices=NCORES, use_collective=True, repeat=1):
    """Build the SPMD Bass graph (same program on all 8 cores).

    n_devices=1 + use_collective=False builds a single-core variant (the
    AllGather replaced by local DMA copies of equivalent volume) for
    TimelineSim cost-model analysis.  repeat=K unrolls the whole pipeline K
    times inside one NEFF for steady-state timing (dispatch amortized).
    """
    key = (L, n_devices, use_collective, repeat)
    if key in _BUILD_CACHE:
        return _BUILD_CACHE[key]

    import concourse.bass as bass  # noqa: F401
    import concourse.mybir as mybir
    import concourse.tile as tile
    from concourse import bacc

    f32 = mybir.dt.float32
    bf16 = mybir.dt.bfloat16

    NI = min(512, L)  # i-chunk (moving free dim)
    ICN = L // NI
    JT = L // P  # j tiles (key/value rows)
    EB = D // P  # e blocks (contraction over d_model)
    DT = DG // P  # d tiles in this core's slice = 2
    GS = min(4, JT)  # score psum group (banks per exp call)
    NGRP = JT // GS

    assert L % NI == 0 and L % P == 0

    nc = bacc.Bacc("TRN2", target_bir_lowering=False, debug=False, num_devices=n_devices)

    xq_d = nc.dram_tensor("xq", [L, D], f32, kind="ExternalInput")
    xk_d = nc.dram_tensor("xk", [L, D], f32, kind="ExternalInput")
    xv_d = nc.dram_tensor("xv", [L, D], f32, kind="ExternalInput")
    wq_d = nc.dram_tensor("wq", [DG, D], f32, kind="ExternalInput")
    wk_d = nc.dram_tensor("wk", [DG, D], f32, kind="ExternalInput")
    wv_d = nc.dram_tensor("wv", [DG, D], f32, kind="ExternalInput")
    bq_d = nc.dram_tensor("bq", [DG], f32, kind="ExternalInput")
    wo_d = nc.dram_tensor("wo", [DG, D], f32, kind="ExternalInput")
    bo_d = nc.dram_tensor("bo", [DG], f32, kind="ExternalInput")
    y_d = nc.dram_tensor("y", [L, DG], f32, kind="ExternalOutput")

    with tile.TileContext(nc) as tc:
        with (
            tc.tile_pool(name="const", bufs=1) as constp,
            tc.tile_pool(name="dram", bufs=1, space="DRAM") as dram,
            tc.tile_pool(name="proj_out", bufs=1) as projout,
            tc.tile_pool(name="norm", bufs=2) as npool,
        ):
            # ---- constants
            ones_k1 = constp.tile([1, P], bf16, name="ones_k1")
            nc.gpsimd.memset(ones_k1[:], 1.0)
            bq_sb = constp.tile([P, DT], f32, name="bq_sb")
            nc.sync.dma_start(bq_sb[:], bq_d.ap().rearrange("(dt p) -> p dt", p=P))
            bo_sb = constp.tile([1, DG], bf16, name="bo_sb")
            nc.gpsimd.dma_start(bo_sb[:], bo_d.ap().rearrange("(o d) -> o d", o=1))

            # ---- bf16 staging in DRAM (cast on SWDGE) for the transposed loads
            # per-e-block contiguous staging: cast(eb) -> transpose(eb) pipeline
            EB_ = D // P
            xq_bf = dram.tile([EB_, L, P], bf16, name="xq_bf")
            xk_bf = dram.tile([EB_, L, P], bf16, name="xk_bf")
            xv_bf = dram.tile([EB_, L, P], bf16, name="xv_bf")
            wq_bf = dram.tile([EB_, DG, P], bf16, name="wq_bf")
            wk_bf = dram.tile([EB_, DG, P], bf16, name="wk_bf")
            wv_bf = dram.tile([EB_, DG, P], bf16, name="wv_bf")
            wo_bf = dram.tile([EB_, DG, P], bf16, name="wo_bf")

            # ---- projection outputs (live into attention phase)
            qt_sb = projout.tile([P, DT, L], bf16, name="qt_sb")  # QT[d, i]
            kt_sb = projout.tile([P, DT, L], bf16, name="kt_sb")  # KT[d, j]
            v_sb = projout.tile([P, JT, HL, DH + 1], bf16, name="v_sb")  # V + ones col
            nc.gpsimd.memset(v_sb[:, :, :, DH : DH + 1], 1.0)
            xtp_sb = projout.tile([P, DT, L], bf16, name="xtp_sb")  # attn out.T

            for _rep in range(repeat):
                _emit_body(
                    nc, tc, mybir, bf16, f32,
                    L, NI, ICN, JT, EB, DT, GS, NGRP, use_collective,
                    xq_d, xk_d, xv_d, wq_d, wk_d, wv_d, wo_d, y_d,
                    xq_bf, xk_bf, xv_bf, wq_bf, wk_bf, wv_bf, wo_bf,
                    ones_k1, bq_sb, bo_sb, qt_sb, kt_sb, v_sb, xtp_sb,
                    dram, npool,
                )

    nc.compile()
    _BUILD_CACHE[key] = nc
    return nc


def _emit_body(
    nc, tc, mybir, bf16, f32,
    L, NI, ICN, JT, EB, DT, GS, NGRP, use_collective,
    xq_d, xk_d, xv_d, wq_d, wk_d, wv_d, wo_d, y_d,
    xq_bf, xk_bf, xv_bf, wq_bf, wk_bf, wv_bf, wo_bf,
    ones_k1, bq_sb, bo_sb, qt_sb, kt_sb, v_sb, xtp_sb,
    dram, npool,
):
    # ---- casts: per-e-block column slices (cast eb -> transpose eb pipeline)
    for src_, dst in (
        (xq_d, xq_bf), (xk_d, xk_bf), (xv_d, xv_bf),
        (wq_d, wq_bf), (wk_d, wk_bf), (wv_d, wv_bf), (wo_d, wo_bf),
    ):
        for eb in range(EB):
            nc.gpsimd.dma_start(
                dst[eb, :, :], src_.ap()[:, eb * P : (eb + 1) * P]
            )

    # ================= phase A: transposed loads + projections ========
    with (
        tc.tile_pool(name="xt", bufs=1) as xtp,
        tc.tile_pool(name="wt", bufs=1) as wtp,
        tc.tile_pool(name="ppsum", bufs=3, space="PSUM") as ppool,
    ):
        xqt = xtp.tile([P, EB, L], bf16, name="xqt")
        xkt = xtp.tile([P, EB, L], bf16, name="xkt")
        xvt = xtp.tile([P, EB, L], bf16, name="xvt")
        wqt = wtp.tile([P, EB, DG], bf16, name="wqt")
        wkt = wtp.tile([P, EB, DG], bf16, name="wkt")
        wvt = wtp.tile([P, EB, DG], bf16, name="wvt")
        for bf, xt in ((xk_bf, xkt), (xv_bf, xvt), (xq_bf, xqt)):
            for eb in range(EB):
                nc.sync.dma_start_transpose(xt[:, eb, :], bf[eb, :, :])
        for bf, wt in ((wk_bf, wkt), (wv_bf, wvt), (wq_bf, wqt)):
            for eb in range(EB):
                nc.sync.dma_start_transpose(wt[:, eb, :], bf[eb, :, :])

        # KT = wk @ xk.T ; QT = wq @ xq.T (+bq)
        for wt, xt, out, bias in (
            (wkt, xkt, kt_sb, False),
            (wqt, xqt, qt_sb, True),
        ):
            for dt in range(DT):
                for ic in range(ICN):
                    ps = ppool.tile([P, NI], f32, tag="pp", name="ps_proj")
                    for eb in range(EB):
                        nc.tensor.matmul(
                            ps[:],
                            wt[:, eb, dt * P : (dt + 1) * P],
                            xt[:, eb, ic * NI : (ic + 1) * NI],
                            start=(eb == 0),
                            stop=(eb == EB - 1),
                        )
                    dst = out[:, dt, ic * NI : (ic + 1) * NI]
                    if bias:
                        nc.scalar.add(dst, ps[:], bq_sb[:, dt : dt + 1])
                    else:
                        nc.vector.tensor_copy(dst, ps[:])

        # V natural: V[j, d] for the 4 local heads
        for jt in range(JT):
            ps = ppool.tile([P, NI], f32, tag="pp", name="ps_vproj")
            for eb in range(EB):
                nc.tensor.matmul(
                    ps[:, 0:DG],
                    xvt[:, eb, jt * P : (jt + 1) * P],
                    wvt[:, eb, :],
                    start=(eb == 0),
                    stop=(eb == EB - 1),
                )
            nc.vector.tensor_copy(
                v_sb[:, jt, :, 0:DH],
                ps[:, 0:DG].rearrange("p (h d) -> p h d", h=HL),
            )

    # ================= phase B: attention + per-chunk AllGather + Wo ====
    with (
        tc.tile_pool(name="attn", bufs=1) as attnp,
        tc.tile_pool(name="et_pool", bufs=2) as etp,
        tc.tile_pool(name="recv_pool", bufs=2) as recvp,
        tc.tile_pool(name="y_pool", bufs=2) as yp,
        tc.tile_pool(name="spsum", bufs=1, space="PSUM") as spool,
        tc.tile_pool(name="avpsum", bufs=2, space="PSUM") as avpool,
        tc.tile_pool(name="wopsum", bufs=2, space="PSUM") as wopool,
    ):
        wot = attnp.tile([P, EB, DG], bf16, name="wot")
        for cb in range(EB):
            nc.sync.dma_start_transpose(wot[:, cb, :], wo_bf[cb, :, :])

        for ic in range(ICN):
            for h in range(HL):
                hp = 64 * (h % 2)  # partition base of this head's 64 dims
                hdt = h // 2
                et = etp.tile([P, JT, NI], bf16, tag="et", name="et")
                for u in range(NGRP):
                    ps4 = spool.tile([P, GS, NI], f32, tag="sc", name="ps_sc")
                    for jj in range(GS):
                        jt = u * GS + jj
                        nc.tensor.matmul(
                            ps4[:, jj, :],
                            kt_sb[hp : hp + DH, hdt, jt * P : (jt + 1) * P],
                            qt_sb[hp : hp + DH, hdt, ic * NI : (ic + 1) * NI],
                            start=True,
                            stop=True,
                        )
                    nc.scalar.activation(
                        et[:, u * GS : (u + 1) * GS, :],
                        ps4[:],
                        mybir.ActivationFunctionType.Exp,
                        scale=SCALE,
                    )
                pso = avpool.tile([P, NI], f32, tag="av", name="ps_av")
                for jt in range(JT):
                    nc.tensor.matmul(
                        pso[0 : DH + 1, :],
                        v_sb[:, jt, h, :],
                        et[:, jt, :],
                        start=(jt == 0),
                        stop=(jt == JT - 1),
                    )
                # reciprocal lane-shifts p64 -> p0 (HW-verified); the
                # gpsimd broadcast source must sit at partition 0.
                rinv = npool.tile([1, NI], f32, tag="rinv", name="rinv")
                nc.vector.reciprocal(rinv[:], pso[DH : DH + 1, :])
                bc = npool.tile([DH, NI], f32, tag="bc", name="bc")
                nc.gpsimd.partition_broadcast(bc[:], rinv[:])
                nc.vector.tensor_mul(
                    xtp_sb[hp : hp + DH, hdt, ic * NI : (ic + 1) * NI],
                    pso[0:DH, :],
                    bc[:],
                )

            # ---- AllGather this i-chunk across the 4-core batch group
            ag_in = dram.tile([DG, NI], bf16, tag="ag_in", bufs=2, name="ag_in")
            ag_out = dram.tile([D, NI], bf16, tag="ag_out", bufs=2, name="ag_out")
            for dt in range(DT):
                nc.sync.dma_start(
                    ag_in[dt * P : (dt + 1) * P, :],
                    xtp_sb[:, dt, ic * NI : (ic + 1) * NI],
                )
            if use_collective:
                nc.gpsimd.collective_compute(
                    "AllGather",
                    mybir.AluOpType.bypass,
                    replica_groups=REPLICA_GROUPS,
                    ins=[ag_in.opt()],
                    outs=[ag_out.opt()],
                )
            else:  # timing stand-in: same bytes moved, no collective
                for r in range(G):
                    nc.sync.dma_start(ag_out[r * DG : (r + 1) * DG, :], ag_in[:])
            recv_sb = recvp.tile([P, EB, NI], bf16, tag="recv", name="recv_sb")
            nc.sync.dma_start(
                recv_sb[:], ag_out.rearrange("(cc p) i -> p cc i", p=P)
            )

            # ---- y rows of this i-chunk (column shard of full y)
            y_sb = yp.tile([P, NI // P, DG], f32, tag="ysb", name="y_sb")
            for itl in range(NI // P):
                psy = wopool.tile([P, DG], f32, tag="wo", name="ps_y")
                for cc in range(EB):
                    nc.tensor.matmul(
                        psy[:],
                        recv_sb[:, cc, itl * P : (itl + 1) * P],
                        wot[:, cc, :],
                        start=(cc == 0),
                        stop=False,
                    )
                nc.tensor.matmul(
                    psy[:],
                    ones_k1[0:1, :],
                    bo_sb[0:1, :],
                    start=False,
                    stop=True,
                )
                nc.vector.tensor_copy(y_sb[:, itl, :], psy[:])
            nc.sync.dma_start(
                y_d.ap()[ic * NI : (ic + 1) * NI, :].rearrange(
                    "(it p) d -> p it d", p=P
                ),
                y_sb[:],
            )


def shard_inputs(q, k, v, Wq, bq, Wk, bk, Wv, bv, Wo, bo, L=L_FULL):
    """Full inputs -> per-core in_maps (list of 8 dicts)."""
    f = lambda a: np.ascontiguousarray(np.asarray(a, dtype=np.float32))
    q, k, v = f(q), f(k), f(v)
    Wq, Wk, Wv, Wo = f(Wq), f(Wk), f(Wv), f(Wo)
    bq, bk, bv, bo = f(bq), f(bk), f(bv), f(bo)
    bo_eff = bo + Wo @ bv  # exact: V-bias and output-bias fold (see header)
    in_maps = []
    for c in range(NCORES):
        b, g = c // G, c % G
        sl = slice(g * DG, (g + 1) * DG)
        in_maps.append(
            {
                "xq": q[b],
                "xk": k[b],
                "xv": v[b],
                "wq": Wq[sl],
                "wk": Wk[sl],
                "wv": Wv[sl],
                "bq": bq[sl],
                "wo": np.ascontiguousarray(Wo[sl]),
                "bo": np.ascontiguousarray(bo_eff[sl]),
            }
        )
    return in_maps


def assemble_output(results, L=L_FULL):
    """Per-core column shards -> full (B, L, D) output."""
    y = np.empty((B, L, D), dtype=np.float32)
    for c in range(NCORES):
        b, g = c // G, c % G
        y[b, :, g * DG : (g + 1) * DG] = results[c]["y"]
    return y


def kernel(q, k, v, Wq, bq, Wk, bk, Wv, bv, Wo, bo):
    from concourse.bass_utils import run_bass_kernel_spmd

    nc = build_kernel(L_FULL)
    in_maps = shard_inputs(q, k, v, Wq, bq, Wk, bk, Wv, bv, Wo, bo)
    res = run_bass_kernel_spmd(nc, in_maps, core_ids=list(range(NCORES)))
    return assemble_output(res.results)


# revision 32
# speedup vs baseline: 4.9992x; 4.9992x over previous
"""Multi-head attention (B=2, L=2048, D=1024, H=16) on 8 trn2 NeuronCores.

Sharding: core c -> batch b = c//4, head-group g = c%4 (4 heads each).
Each core projects Q/K/V for its 256-wide d_model slice, runs attention for
its 4 heads, then an AllGather inside each 4-core batch group collects the
per-head-group attention outputs (as X.T, concat-dim on partitions) and every
core computes a distinct 256-column shard of the final Wo projection (its
own Wo row-slice arrives as per-core input data, so all addressing is
static).  The host stacks the column shards.

Math notes (exactness-preserving simplifications):
  - K-projection bias bk adds a per-query constant to every score row and
    cancels exactly in softmax -> dropped on device.
  - V bias bv contributes (sum_j P[i,j]) * bv = bv after normalization, so
    y += bv @ Wo.T; it is folded into bo on the host (bo_eff = bo + Wo @ bv).
  - Scores never exceed ~|2.5| for these input scales -> softmax runs
    without max subtraction (mathematically identical).
Compute is bf16 into fp32 PSUM accumulation throughout.
"""

import sys

if "/opt/trn_rl_repo" not in sys.path:
    sys.path.insert(0, "/opt/trn_rl_repo")

import numpy as np

B, L_FULL, D, H = 2, 2048, 1024, 16
G = 4  # cores per batch group (head parallel)
NCORES = 8
HL = H // G  # heads per core = 4
DH = D // H  # 64
DG = D // G  # d_model slice per core = 256
P = 128
REPLICA_GROUPS = [[0, 1, 2, 3], [4, 5, 6, 7]]
SCALE = 1.0 / float(np.sqrt(DH))

_BUILD_CACHE = {}


def build_kernel(L=L_FULL, n_devices=NCORES, use_collective=True, repeat=1, stage="all"):
    """Build the SPMD Bass graph (same program on all 8 cores).

    n_devices=1 + use_collective=False builds a single-core variant (the
    AllGather replaced by local DMA copies of equivalent volume) for
    TimelineSim cost-model analysis.  repeat=K unrolls the whole pipeline K
    times inside one NEFF for steady-state timing (dispatch amortized).
    """
    key = (L, n_devices, use_collective, repeat, stage)
    if key in _BUILD_CACHE:
        return _BUILD_CACHE[key]

    import concourse.bass as bass  # noqa: F401
    from concourse.masks import make_identity
    import concourse.mybir as mybir
    import concourse.tile as tile
    from concourse import bacc

    f32 = mybir.dt.float32
    bf16 = mybir.dt.bfloat16

    NI = min(512, L)  # i-chunk (moving free dim)
    ICN = L // NI
    JT = L // P  # j tiles (key/value rows)
    EB = D // P  # e blocks (contraction over d_model)
    DT = DG // P  # d tiles in this core's slice = 2
    GS = min(4, JT)  # score psum group (banks per exp call)
    NGRP = JT // GS
    # 2 slots of <=3 banks pipeline PE (scores) against ACT (exp)
    GROUPS_SCHED = []
    rem = JT
    while rem > 4:
        GROUPS_SCHED.append(3)
        rem -= 3
    GROUPS_SCHED.extend({4: [2, 2], 3: [3], 2: [2], 1: [1], 0: []}[rem])
    GSMAX = max(GROUPS_SCHED)

    assert L % NI == 0 and L % P == 0

    nc = bacc.Bacc("TRN2", target_bir_lowering=False, debug=False, num_devices=n_devices)

    xq_d = nc.dram_tensor("xq", [L, D], f32, kind="ExternalInput")
    xk_d = nc.dram_tensor("xk", [L, D], f32, kind="ExternalInput")
    xv_d = nc.dram_tensor("xv", [L, D], f32, kind="ExternalInput")
    wq_d = nc.dram_tensor("wq", [DG, D], f32, kind="ExternalInput")
    wk_d = nc.dram_tensor("wk", [DG, D], f32, kind="ExternalInput")
    wv_d = nc.dram_tensor("wv", [DG, D], f32, kind="ExternalInput")
    bq_d = nc.dram_tensor("bq", [DG], f32, kind="ExternalInput")
    wo_d = nc.dram_tensor("wo", [DG, D], f32, kind="ExternalInput")
    bo_d = nc.dram_tensor("bo", [DG], f32, kind="ExternalInput")
    y_d = nc.dram_tensor("y", [L, DG], f32, kind="ExternalOutput")

    with tile.TileContext(nc) as tc:
        with (
            tc.tile_pool(name="const", bufs=1) as constp,
            tc.tile_pool(name="dram", bufs=1, space="DRAM") as dram,
            tc.tile_pool(name="proj_out", bufs=1) as projout,
            tc.tile_pool(name="norm", bufs=2) as npool,
        ):
            # ---- constants
            ones_k1 = constp.tile([1, P], bf16, name="ones_k1")
            nc.gpsimd.memset(ones_k1[:], 1.0)
            ident = constp.tile([P, P], bf16, name="ident")
            make_identity(nc, ident[:])
            bq_sb = constp.tile([P, DT], f32, name="bq_sb")
            nc.sync.dma_start(bq_sb[:], bq_d.ap().rearrange("(dt p) -> p dt", p=P))
            bo_sb = constp.tile([1, DG], bf16, name="bo_sb")
            nc.gpsimd.dma_start(bo_sb[:], bo_d.ap().rearrange("(o d) -> o d", o=1))

            # ---- bf16 staging in DRAM (xbar transpose path: xv, xq only;
            #      xk and the weights go through PE transposes instead)
            xq_bf = dram.tile([L, D], bf16, name="xq_bf")
            xv_bf = dram.tile([L, D], bf16, name="xv_bf")
            wot_sb = projout.tile([P, D // P, DG], bf16, name="wot_sb")

            # ---- projection outputs (live into attention phase)
            qt_sb = projout.tile([P, DT, L], bf16, name="qt_sb")  # QT[d, i]
            kt_sb = projout.tile([P, DT, L], bf16, name="kt_sb")  # KT[d, j]
            v_sb = projout.tile([P, JT, HL, DH + 1], bf16, name="v_sb")  # V + ones col
            nc.gpsimd.memset(v_sb[:, :, :, DH : DH + 1], 1.0)
            xtp_sb = projout.tile([P, DT, L], bf16, name="xtp_sb")  # attn out.T

            for _rep in range(repeat):
                _emit_body(
                    nc, tc, mybir, bf16, f32,
                    L, NI, ICN, JT, EB, DT, GROUPS_SCHED, GSMAX, use_collective,
                    xq_d, xk_d, xv_d, wq_d, wk_d, wv_d, wo_d, y_d,
                    xq_bf, xv_bf, ident, wot_sb,
                    ones_k1, bq_sb, bo_sb, qt_sb, kt_sb, v_sb, xtp_sb,
                    dram, npool, stage,
                )

    nc.compile()
    _BUILD_CACHE[key] = nc
    return nc


def _emit_body(
    nc, tc, mybir, bf16, f32,
    L, NI, ICN, JT, EB, DT, GROUPS_SCHED, GSMAX, use_collective,
    xq_d, xk_d, xv_d, wq_d, wk_d, wv_d, wo_d, y_d,
    xq_bf, xv_bf, ident, wot_sb,
    ones_k1, bq_sb, bo_sb, qt_sb, kt_sb, v_sb, xtp_sb,
    dram, npool, stage="all",
):
    # ---- casts for the xbar path: contiguous row chunks (v then q)
    for src_, dst in ((xv_d, xv_bf), (xq_d, xq_bf)):
        rc = L // 8
        for i in range(8):
            nc.gpsimd.dma_start(
                dst[i * rc : (i + 1) * rc, :], src_.ap()[i * rc : (i + 1) * rc, :]
            )

    if stage == "casts":
        return
    # ================= phase A: transposed loads + projections ========
    with (
        tc.tile_pool(name="xt", bufs=1) as xtp,
        tc.tile_pool(name="wt", bufs=1) as wtp,
        tc.tile_pool(name="nat", bufs=3) as natp,
        tc.tile_pool(name="ppsum", bufs=3, space="PSUM") as ppool,
        tc.tile_pool(name="tpsum", bufs=2, space="PSUM") as tpool,
    ):
        xqt = xtp.tile([P, EB, L], bf16, name="xqt")
        xkt = xtp.tile([P, EB, L], bf16, name="xkt")
        xvt = xtp.tile([P, EB, L], bf16, name="xvt")
        wqt = wtp.tile([P, EB, DG], bf16, name="wqt")
        wkt = wtp.tile([P, EB, DG], bf16, name="wkt")
        wvt = wtp.tile([P, EB, DG], bf16, name="wvt")

        # -- PE-transpose path: xk + all weights (cast-DMA straight to SBUF,
        #    8 transposes batched into one 2-bank psum, one DVE copy out)
        def pe_transpose(src_d, rows, out_t, out_col0):
            # src rows [out_col0*P ...] of a [rows, D] f32 DRAM tensor
            nat = natp.tile([P, D], bf16, tag="nat", name="nat")
            nc.gpsimd.dma_start(
                nat[:], src_d.ap()[out_col0 * P : (out_col0 + 1) * P, :]
            )
            pst = tpool.tile([P, EB, P], bf16, tag="tp", name="ps_tr")
            for eb in range(EB):
                nc.tensor.transpose(
                    pst[:, eb, :], nat[:, eb * P : (eb + 1) * P], ident[:]
                )
            nc.vector.tensor_copy(
                out_t[:, :, out_col0 * P : (out_col0 + 1) * P], pst[:]
            )

        for it in range(L // P):
            pe_transpose(xk_d, L, xkt, it)
        for w_d, wt in ((wk_d, wkt), (wv_d, wvt), (wq_d, wqt), (wo_d, wot_sb)):
            for dt in range(DT):
                pe_transpose(w_d, DG, wt, dt)

        # -- xbar path: xv, xq
        for bf, xt in ((xv_bf, xvt), (xq_bf, xqt)):
            for eb in range(EB):
                nc.sync.dma_start_transpose(
                    xt[:, eb, :], bf[0:L, eb * P : (eb + 1) * P]
                )

        if stage == "loads":
            return
        # KT = wk @ xk.T ; QT = wq @ xq.T (+bq)
        for wt, xt, out, bias in (
            (wkt, xkt, kt_sb, False),
            (wqt, xqt, qt_sb, True),
        ):
            for dt in range(DT):
                for ic in range(ICN):
                    ps = ppool.tile([P, NI], f32, tag="pp", name="ps_proj")
                    for eb in range(EB):
                        nc.tensor.matmul(
                            ps[:],
                            wt[:, eb, dt * P : (dt + 1) * P],
                            xt[:, eb, ic * NI : (ic + 1) * NI],
                            start=(eb == 0),
                            stop=(eb == EB - 1),
                        )
                    dst = out[:, dt, ic * NI : (ic + 1) * NI]
                    if bias:
                        nc.scalar.add(dst, ps[:], bq_sb[:, dt : dt + 1])
                    else:
                        nc.vector.tensor_copy(dst, ps[:])

        # V natural: V[j, d] for the 4 local heads
        for jt in range(JT):
            ps = ppool.tile([P, NI], f32, tag="pp", name="ps_vproj")
            for eb in range(EB):
                nc.tensor.matmul(
                    ps[:, 0:DG],
                    xvt[:, eb, jt * P : (jt + 1) * P],
                    wvt[:, eb, :],
                    start=(eb == 0),
                    stop=(eb == EB - 1),
                )
            nc.vector.tensor_copy(
                v_sb[:, jt, :, 0:DH],
                ps[:, 0:DG].rearrange("p (h d) -> p h d", h=HL),
            )

    if stage == "proj":
        return
    # ================= phase B: attention + per-chunk AllGather + Wo ====
    with (
        tc.tile_pool(name="attn", bufs=1) as attnp,
        tc.tile_pool(name="et_pool", bufs=2) as etp,
        tc.tile_pool(name="recv_pool", bufs=2) as recvp,
        tc.tile_pool(name="y_pool", bufs=2) as yp,
        tc.tile_pool(name="spsum", bufs=2, space="PSUM") as spool,
        tc.tile_pool(name="avpsum", bufs=1, space="PSUM") as avpool,
        tc.tile_pool(name="wopsum", bufs=1, space="PSUM") as wopool,
    ):
        wot = wot_sb

        for ic in range(ICN):
            for h in range(HL):
                hp = 64 * (h % 2)  # partition base of this head's 64 dims
                hdt = h // 2
                et = etp.tile([P, JT, NI], bf16, tag="et", name="et")
                jt0 = 0
                for gs in GROUPS_SCHED:
                    ps4 = spool.tile([P, GSMAX, NI], f32, tag="sc", name="ps_sc")
                    for jj in range(gs):
                        jt = jt0 + jj
                        nc.tensor.matmul(
                            ps4[:, jj, :],
                            kt_sb[hp : hp + DH, hdt, jt * P : (jt + 1) * P],
                            qt_sb[hp : hp + DH, hdt, ic * NI : (ic + 1) * NI],
                            start=True,
                            stop=True,
                        )
                    nc.scalar.activation(
                        et[:, jt0 : jt0 + gs, :],
                        ps4[:, 0:gs, :],
                        mybir.ActivationFunctionType.Exp,
                        scale=SCALE,
                    )
                    jt0 += gs
                pso = avpool.tile([P, NI], f32, tag="av", name="ps_av")
                for jt in range(JT):
                    nc.tensor.matmul(
                        pso[0 : DH + 1, :],
                        v_sb[:, jt, h, :],
                        et[:, jt, :],
                        start=(jt == 0),
                        stop=(jt == JT - 1),
                    )
                # reciprocal lane-shifts p64 -> p0 (HW-verified); the
                # gpsimd broadcast source must sit at partition 0.
                rinv = npool.tile([1, NI], f32, tag="rinv", name="rinv")
                nc.vector.reciprocal(rinv[:], pso[DH : DH + 1, :])
                bc = npool.tile([DH, NI], f32, tag="bc", name="bc")
                nc.gpsimd.partition_broadcast(bc[:], rinv[:])
                nc.vector.tensor_mul(
                    xtp_sb[hp : hp + DH, hdt, ic * NI : (ic + 1) * NI],
                    pso[0:DH, :],
                    bc[:],
                )

            if stage == "attn":
                continue
            # ---- AllGather this i-chunk across the 4-core batch group
            ag_in = dram.tile([DG, NI], bf16, tag="ag_in", bufs=2, name="ag_in")
            ag_out = dram.tile([D, NI], bf16, tag="ag_out", bufs=2, name="ag_out")
            for dt in range(DT):
                nc.sync.dma_start(
                    ag_in[dt * P : (dt + 1) * P, :],
                    xtp_sb[:, dt, ic * NI : (ic + 1) * NI],
                )
            if use_collective:
                nc.gpsimd.collective_compute(
                    "AllGather",
                    mybir.AluOpType.bypass,
                    replica_groups=REPLICA_GROUPS,
                    ins=[ag_in.opt()],
                    outs=[ag_out.opt()],
                )
            else:  # timing stand-in: same bytes moved, no collective
                for r in range(G):
                    nc.sync.dma_start(ag_out[r * DG : (r + 1) * DG, :], ag_in[:])
            recv_sb = recvp.tile([P, EB, NI], bf16, tag="recv", name="recv_sb")
            nc.sync.dma_start(
                recv_sb[:], ag_out.rearrange("(cc p) i -> p cc i", p=P)
            )

            # ---- y rows of this i-chunk (column shard of full y)
            y_sb = yp.tile([P, NI // P, DG], f32, tag="ysb", name="y_sb")
            for itl in range(NI // P):
                psy = wopool.tile([P, DG], f32, tag="wo", name="ps_y")
                for cc in range(EB):
                    nc.tensor.matmul(
                        psy[:],
                        recv_sb[:, cc, itl * P : (itl + 1) * P],
                        wot[:, cc, :],
                        start=(cc == 0),
                        stop=False,
                    )
                nc.tensor.matmul(
                    psy[:],
                    ones_k1[0:1, :],
                    bo_sb[0:1, :],
                    start=False,
                    stop=True,
                )
                nc.vector.tensor_copy(y_sb[:, itl, :], psy[:])
            nc.sync.dma_start(
                y_d.ap()[ic * NI : (ic + 1) * NI, :].rearrange(
                    "(it p) d -> p it d", p=P
                ),
                y_sb[:],
            )


def shard_inputs(q, k, v, Wq, bq, Wk, bk, Wv, bv, Wo, bo, L=L_FULL):
    """Full inputs -> per-core in_maps (list of 8 dicts)."""
    f = lambda a: np.ascontiguousarray(np.asarray(a, dtype=np.float32))
    q, k, v = f(q), f(k), f(v)
    Wq, Wk, Wv, Wo = f(Wq), f(Wk), f(Wv), f(Wo)
    bq, bk, bv, bo = f(bq), f(bk), f(bv), f(bo)
    bo_eff = bo + Wo @ bv  # exact: V-bias and output-bias fold (see header)
    in_maps = []
    for c in range(NCORES):
        b, g = c // G, c % G
        sl = slice(g * DG, (g + 1) * DG)
        in_maps.append(
            {
                "xq": q[b],
                "xk": k[b],
                "xv": v[b],
                "wq": Wq[sl],
                "wk": Wk[sl],
                "wv": Wv[sl],
                "bq": bq[sl],
                "wo": np.ascontiguousarray(Wo[sl]),
                "bo": np.ascontiguousarray(bo_eff[sl]),
            }
        )
    return in_maps


def assemble_output(results, L=L_FULL):
    """Per-core column shards -> full (B, L, D) output."""
    y = np.empty((B, L, D), dtype=np.float32)
    for c in range(NCORES):
        b, g = c // G, c % G
        y[b, :, g * DG : (g + 1) * DG] = results[c]["y"]
    return y


def kernel(q, k, v, Wq, bq, Wk, bk, Wv, bv, Wo, bo):
    from concourse.bass_utils import run_bass_kernel_spmd

    nc = build_kernel(L_FULL)
    in_maps = shard_inputs(q, k, v, Wq, bq, Wk, bk, Wv, bv, Wo, bo)
    res = run_bass_kernel_spmd(nc, in_maps, core_ids=list(range(NCORES)))
    return assemble_output(res.results)
